# revision 25
# baseline (speedup 1.0000x reference)
"""Mixtral attention block (B=1, S=2048, D=4096, H=32, KVH=8, HD=128) on 8
Trainium2 NeuronCores.

Sharding: tensor-parallel by heads. Core c owns q-heads [4c, 4c+4) (columns
512c:512c+512 of wq), kv-head c (columns 128c:128c+128 of wk/wv) and the
matching rows 512c:512c+512 of wo. Each core computes a partial (S, D) output
through its wo row-slice; the host sums the 8 partials (the unshard step for
row-parallel wo) and reshapes to (1, S, D).

Device kernel layout choices (per core):
  - hidden_states is passed transposed (xT, [D, S]) in bf16 so all matmuls
    have the contraction dim on partitions.
  - Phase 1 computes Q^T [hd, S] / K^T [hd, S] (weight chunk stationary) and
    V in natural [S, hd] layout (xT chunk stationary), RoPE applied on the
    fp32 PSUM accumulators.
  - Phase 2 computes scores transposed, S^T[k, q] = (K^T chunk).T @ Q^T, so
    the softmax reduction over k is a partition-dim reduction done on the PE
    with an all-ones [128,128] stationary operand, which also broadcasts the
    denominators to all partitions; exp on ACT (scale folded in), causal
    masking by multiplying the exp'd diagonal-block tiles with 0/1 masks.
    P^T tiles then feed PV matmuls with V-natural stationary, producing
    A^T [hd, q] directly.
  - Phase 3 multiplies A^T by the broadcast reciprocal denominators and
    feeds wo row-chunks (A^T chunk stationary), accumulating the partial
    output in natural [S, D] layout.

bf16 matmuls with fp32 accumulation everywhere; exp and reciprocal in fp32.
No softmax max-subtraction: scores are O(10) for this problem family, far
from fp32 exp overflow, and softmax is shift-invariant.
attention_mask is all-ones per the problem spec (fill: "ones"), so only the
causal mask is applied.
"""

import sys

if "/opt/trn_rl_repo" not in sys.path:
    sys.path.insert(0, "/opt/trn_rl_repo")

import ast
from contextlib import ExitStack

import numpy as np
import ml_dtypes

import bass_rust
import concourse.bass as bass
import concourse.tile as tile
from concourse import mybir
from concourse.vector_clock import ScopedClock

BF16 = mybir.dt.bfloat16
F32 = mybir.dt.float32

B, S, D = 1, 2048, 4096
H, KVH, HD = 32, 8, 128
N_CORES = 8
QH = H // N_CORES          # 4 q heads per core
LOCAL = QH * HD            # 512 columns of wq / rows of wo per core
SCALE = float(HD) ** -0.5
SC = 512                   # sequence chunk for phase-1/2 free dims
N_SC = S // SC             # 4
N_D = D // 128             # 32 contraction chunks
N_ST = S // 128            # 16 sequence tiles of 128
N_OC = D // 512            # 8 output-column chunks


# ---------------------------------------------------------------------------
# The neuronxcc walrus build used under axon rejects instructions with more
# than one sync wait ("Too many sync wait commands" in setupSyncWait for the
# tail InstDrain). Split the TileContext tail drain into one drain per
# waited-on logical processor.
# ---------------------------------------------------------------------------
def _clock_values(vc) -> list[int]:
    r = repr(vc)
    return ast.literal_eval(r[r.index("[") : r.index("]") + 1])


def _split_drain_and_barrier(self, tick_clock, wait_clock):
    nc = self.nc
    gc = tick_clock.global_clock
    vals = _clock_values(gc)
    nz = [i for i, v in enumerate(vals) if v > 0]
    for i in nz or [None]:
        mask = [0] * len(vals)
        if i is not None:
            mask[i] = vals[i]
        part = gc.elementwise_min(bass_rust.VectorClock(mask))
        drain_inst = nc.sync.drain()
        wait_clock.add_sem_waits(drain_inst.ins, ScopedClock({None: part}))

    nc.all_engine_barrier()
    assert self.sems is not None
    popped = nc._tile_sem_poison_stack.pop()
    assert popped is self._sem_poison
    nc.clear_and_free_semaphores(list(self.sems.allocated().values()))
    nc.all_engine_barrier()


_orig_commit = tile.TileContext._commit_instruction


def _commit_split_waits(self, inst, lazy_reg_writes: bool = True):
    """This ISA build allows at most one sync wait per instruction
    (bass_rust.inst_waits_full). Tile's wait-assignment pass can attach
    several; spill the extras onto same-engine no-ops committed just before,
    which preserves per-engine ordering semantics."""
    si = getattr(inst, "sync_info", None)
    if si is not None and si.on_wait and len(si.on_wait) > 1:
        waits = list(si.on_wait)
        for w in waits[:-1]:
            nop = mybir.InstNoOp(
                name=self.nc.get_next_instruction_name(),
                sync_info=mybir.SyncInfo(on_wait=[w], on_update=[]),
                bass_nofuse=True,
                engine=inst.engine,
            )
            _orig_commit(self, nop, lazy_reg_writes)
        inst.sync_info = mybir.SyncInfo(
            on_wait=[waits[-1]], on_update=list(si.on_update)
        )
    return _orig_commit(self, inst, lazy_reg_writes)


def _apply_tile_patch():
    tile.TileContext._drain_and_barrier = _split_drain_and_barrier
    tile.TileContext._commit_instruction = _commit_split_waits


# ---------------------------------------------------------------------------
# Device kernel
# ---------------------------------------------------------------------------
def build_nc(seq: int = S, reps: int = 1) -> bass.Bass:
    """reps > 1 repeats the whole computation (including input DMAs) inside
    one NEFF; used to measure per-execution device time by differencing
    single-NEFF wall times, cancelling per-execution dispatch overhead.

    Structure: one fused loop over 512-wide sequence chunks. Iteration sc
    runs (a) projections+RoPE for chunk sc, (b) attention for q-chunk sc
    (all local heads; causally it needs only k/v chunks <= sc, all already
    computed), (c) the output projection for rows [512*sc, 512*sc+512).
    Interleaving keeps the PE instruction stream dense (PE executes in
    program order) and overlaps the xT/weight DMAs of later chunks with
    attention compute. PSUM is managed as one 8-slot pool of [128,512]
    fp32 banks shared by all stages."""
    _apply_tile_patch()
    n_sc = seq // SC

    nc = bass.Bass()
    xT = nc.declare_dram_parameter("xT", [D, seq], BF16, isOutput=False)
    wq = nc.declare_dram_parameter("wq", [D, LOCAL], BF16, isOutput=False)
    wk = nc.declare_dram_parameter("wk", [D, HD], BF16, isOutput=False)
    wv = nc.declare_dram_parameter("wv", [D, HD], BF16, isOutput=False)
    wo = nc.declare_dram_parameter("wo", [LOCAL, D], BF16, isOutput=False)
    cosT = nc.declare_dram_parameter("cosT", [HD, seq], F32, isOutput=False)
    sinT = nc.declare_dram_parameter("sinT", [HD, seq], F32, isOutput=False)
    # 0/1 masks for the 4 relative diagonal k-chunks of a 512-wide q chunk:
    # mask01[k, i, q] = 1 if q >= 128*i + k else 0
    mask01 = nc.declare_dram_parameter("mask01", [HD, 4, SC], BF16, isOutput=False)
    ident = nc.declare_dram_parameter("ident", [128, 128], BF16, isOutput=False)
    out = nc.declare_dram_parameter("out", [seq, D], F32, isOutput=True)

    with ExitStack() as ctx:
        tc = ctx.enter_context(tile.TileContext(nc))

        res = ctx.enter_context(tc.tile_pool(name="resident", bufs=1))
        wq_sb = res.tile([128, N_D, LOCAL], BF16, tag="wq")
        wk_sb = res.tile([128, N_D, HD], BF16, tag="wk")
        wv_sb = res.tile([128, N_D, HD], BF16, tag="wv")
        cos_sb = res.tile([HD, seq], F32, tag="cos")
        sin_sb = res.tile([HD, seq], F32, tag="sin")
        mask_sb = res.tile([HD, 4, SC], BF16, tag="mask")
        ones_sb = res.tile([128, 128], BF16, tag="ones")
        id_sb = res.tile([128, 128], BF16, tag="ident")
        qt_sb = [res.tile([HD, seq], BF16, tag=f"qt{j}", name=f"qt{j}") for j in range(QH)]
        kt_sb = res.tile([HD, seq], BF16, tag="kt")
        v_sb = res.tile([128, seq // 128, HD], BF16, tag="v")
        at_sb = [res.tile([HD, seq], BF16, tag=f"at{j}", name=f"at{j}") for j in range(QH)]
        wo_sb = res.tile([128, QH, D], BF16, tag="wo")

        xp = ctx.enter_context(tc.tile_pool(name="xp", bufs=1))
        tp = ctx.enter_context(tc.tile_pool(name="tp", bufs=3))
        pp = ctx.enter_context(tc.tile_pool(name="pp", bufs=3))
        op = ctx.enter_context(tc.tile_pool(name="op", bufs=3))
        ps = ctx.enter_context(tc.tile_pool(name="ps", bufs=8, space="PSUM"))

        wq_r = wq.rearrange("(n p) m -> p n m", p=128)
        wk_r = wk.rearrange("(n p) m -> p n m", p=128)
        wv_r = wv.rearrange("(n p) m -> p n m", p=128)
        wo_r = wo.rearrange("(n p) m -> p n m", p=128)
        x_r = xT.rearrange("(n p) m -> p n m", p=128)

        for _rep in range(reps):
            nc.vector.memset(ones_sb[:], 1.0)

            for sc in range(n_sc):
                # ---- stage A: projections + RoPE for chunk sc ----
                xh = xp.tile([128, N_D, SC], BF16, tag="xh")
                xg = 1 if sc == 0 else 4
                def load_xh_group(g):
                    nc.sync.dma_start(
                        xh[:, g : g + xg, :],
                        x_r[:, g : g + xg, sc * SC : (sc + 1) * SC],
                    )
                if sc > 0:
                    for g in range(0, N_D, xg):
                        load_xh_group(g)

                nc.sync.dma_start(
                    cos_sb[:, sc * SC : (sc + 1) * SC],
                    cosT[:, sc * SC : (sc + 1) * SC],
                )
                nc.sync.dma_start(
                    sin_sb[:, sc * SC : (sc + 1) * SC],
                    sinT[:, sc * SC : (sc + 1) * SC],
                )
                if sc == 0 and _rep == 0:
                    nc.sync.dma_start(mask_sb[:], mask01[:])
                    nc.sync.dma_start(id_sb[:], ident[:])

                q_ps = [ps.tile([128, SC], F32, tag="ps", name=f"qps{j}") for j in range(QH)]
                k_ps = ps.tile([128, SC], F32, tag="ps")
                v_ps = ps.tile([128, SC], F32, tag="ps")
                for d in range(N_D):
                    if sc == 0 and d % xg == 0:
                        load_xh_group(d)
                    if sc == 0 and _rep == 0:
                        # first use of each weight chunk: load it just in time
                        nc.sync.dma_start(wq_sb[:, d, :], wq_r[:, d, :])
                        if d % 4 == 0:
                            nc.sync.dma_start(
                                wk_sb[:, d : d + 4, :], wk_r[:, d : d + 4, :]
                            )
                            nc.sync.dma_start(
                                wv_sb[:, d : d + 4, :], wv_r[:, d : d + 4, :]
                            )
                    xt = xh[:, d, :]
                    first, last = d == 0, d == N_D - 1
                    for j in range(QH):
                        nc.tensor.matmul(
                            q_ps[j][:],
                            wq_sb[:, d, j * HD : (j + 1) * HD],
                            xt,
                            start=first,
                            stop=last,
                        )
                    nc.tensor.matmul(
                        k_ps[:], wk_sb[:, d, :], xt, start=first, stop=last
                    )
                    nc.tensor.matmul(
                        v_ps[:], wv_sb[:, d, :], xt, start=first, stop=last
                    )

                if sc == 0 and _rep == 0:
                    for j in range(QH):
                        nc.sync.dma_start(wo_sb[:, j, :], wo_r[:, j, :])

                # v_ps holds V^T [hd, s]; copy out to SBUF immediately (ahead
                # of the RoPE DVE work) so the PE transposes that follow the
                # projection matmuls in the PE stream aren't stalled, then
                # transpose 128x128 blocks on the PE into natural [s, hd].
                vt = pp.tile([128, SC], BF16, tag="vt")
                nc.scalar.copy(vt[:], v_ps[:])
                for m in range(4):
                    vtr = ps.tile([128, HD], BF16, tag="ps", name="vtr")
                    nc.tensor.transpose(
                        vtr[:], vt[:, m * HD : (m + 1) * HD], id_sb[:]
                    )
                    nc.scalar.copy(v_sb[:, sc * 4 + m, :], vtr[:])

                # RoPE. Stage B's first matmuls need only the fresh q0 chunk
                # (early kc use k/v chunks from previous iterations), so do q0
                # first, then k (needed by the diagonal kc late in the loop).
                cs = cos_sb[:, sc * SC : (sc + 1) * SC]
                sn = sin_sb[:, sc * SC : (sc + 1) * SC]
                for src_ps, dst in [(q_ps[0], qt_sb[0]), (k_ps, kt_sb)] + [
                    (q_ps[j], qt_sb[j]) for j in range(1, QH)
                ]:
                    t1 = tp.tile([HD, SC], F32, tag="t1")
                    t2 = tp.tile([HD, SC], F32, tag="t2")
                    dv = dst[:, sc * SC : (sc + 1) * SC]
                    nc.vector.tensor_mul(t1[:], src_ps[:], cs)
                    nc.vector.tensor_mul(t2[0:64, :], src_ps[64:128, :], sn[0:64, :])
                    nc.vector.tensor_mul(t2[64:128, :], src_ps[0:64, :], sn[64:128, :])
                    nc.vector.tensor_sub(dv[0:64, :], t1[0:64, :], t2[0:64, :])
                    nc.vector.tensor_add(dv[64:128, :], t1[64:128, :], t2[64:128, :])

                # ---- stage B: attention for q-chunk sc, all local heads ----
                qc = sc
                n_k = 4 * qc + 4
                for j in range(QH):
                    ssum = ps.tile([128, SC], F32, tag="ps")
                    pv = ps.tile([128, SC], F32, tag="ps")
                    qt = qt_sb[j][:, qc * SC : (qc + 1) * SC]
                    for kc in range(n_k):
                        st = ps.tile([128, SC], F32, tag="ps")
                        nc.tensor.matmul(
                            st[:],
                            kt_sb[:, kc * 128 : (kc + 1) * 128],
                            qt,
                            start=True,
                            stop=True,
                        )
                        pt = pp.tile([128, SC], BF16, tag="pt")
                        nc.scalar.activation(
                            pt[:], st[:], mybir.ActivationFunctionType.Exp,
                            scale=SCALE,
                        )
                        if kc >= 4 * qc:
                            nc.vector.tensor_mul(
                                pt[:], pt[:], mask_sb[:, kc - 4 * qc, :]
                            )
                        first, last = kc == 0, kc == n_k - 1
                        nc.tensor.matmul(
                            ssum[:], ones_sb[:], pt[:], start=first, stop=last
                        )
                        nc.tensor.matmul(
                            pv[:], v_sb[:, kc, :], pt[:], start=first, stop=last
                        )
                    recip = pp.tile([128, SC], F32, tag="recip")
                    nc.vector.reciprocal(recip[:], ssum[:])
                    nc.vector.tensor_mul(
                        at_sb[j][:, qc * SC : (qc + 1) * SC], pv[:], recip[:]
                    )

                # ---- stage C: output projection for rows of chunk sc ----
                for st_i in range(4 * sc, 4 * sc + 4):
                    for oc in range(N_OC):
                        ops = ps.tile([128, 512], F32, tag="ps")
                        for j in range(QH):
                            nc.tensor.matmul(
                                ops[:],
                                at_sb[j][:, st_i * 128 : (st_i + 1) * 128],
                                wo_sb[:, j, oc * 512 : (oc + 1) * 512],
                                start=(j == 0),
                                stop=(j == QH - 1),
                            )
                        osb = op.tile([128, 512], F32, tag="osb")
                        nc.vector.tensor_copy(osb[:], ops[:])
                        nc.sync.dma_start(
                            out[st_i * 128 : (st_i + 1) * 128, oc * 512 : (oc + 1) * 512],
                            osb[:],
                        )
    return nc


# ---------------------------------------------------------------------------
# Host-side input prep
# ---------------------------------------------------------------------------
def make_masks() -> np.ndarray:
    k = np.arange(HD)[:, None]
    q = np.arange(SC)[None, :]
    m = np.stack([(q >= 128 * i + k) for i in range(4)], axis=1).astype(ml_dtypes.bfloat16)
    return m


def make_in_maps(hidden_states, cos, sin, wq, wk, wv, wo, seq: int = S):
    bf = ml_dtypes.bfloat16
    x = np.asarray(hidden_states, np.float32).reshape(seq, D)
    xT = np.ascontiguousarray(x.T).astype(bf)
    cosT = np.ascontiguousarray(np.asarray(cos, np.float32).reshape(seq, HD).T)
    sinT = np.ascontiguousarray(np.asarray(sin, np.float32).reshape(seq, HD).T)
    masks = make_masks()
    wqf = np.asarray(wq, np.float32)
    wkf = np.asarray(wk, np.float32)
    wvf = np.asarray(wv, np.float32)
    wof = np.asarray(wo, np.float32)
    in_maps = []
    for c in range(N_CORES):
        in_maps.append(
            {
                "xT": xT,
                "wq": np.ascontiguousarray(wqf[:, c * LOCAL : (c + 1) * LOCAL]).astype(bf),
                "wk": np.ascontiguousarray(wkf[:, c * HD : (c + 1) * HD]).astype(bf),
                "wv": np.ascontiguousarray(wvf[:, c * HD : (c + 1) * HD]).astype(bf),
                "wo": np.ascontiguousarray(wof[c * LOCAL : (c + 1) * LOCAL, :]).astype(bf),
                "cosT": cosT,
                "sinT": sinT,
                "mask01": masks,
                "ident": np.eye(128, dtype=np.float32).astype(bf),
            }
        )
    return in_maps


_CACHE: dict = {}


def _get_runner(reps: int = 1):
    """Compile the device program once; return a callable over device-resident
    sharded inputs. Mirrors concourse.bass2jax.run_bass_via_pjrt's multi-core
    branch, but keeps the jitted executable and lets us re-run for timing."""
    rkey = f"runner{reps}"
    if rkey in _CACHE:
        return _CACHE[rkey]

    import jax
    import jax.numpy as jnp
    from jax.experimental.shard_map import shard_map
    from jax.sharding import Mesh, NamedSharding, PartitionSpec
    from concourse import bass2jax, mybir as _mybir

    bass2jax.install_neuronx_cc_hook()
    nc = _CACHE.setdefault(f"nc{reps}", build_nc(S, reps))

    partition_name = nc.partition_id_tensor.name if nc.partition_id_tensor else None
    in_names: list[str] = []
    out_names: list[str] = []
    out_avals: list = []
    for alloc in nc.m.functions[0].allocations:
        if not isinstance(alloc, _mybir.MemoryLocationSet):
            continue
        name = alloc.memorylocations[0].name
        if alloc.kind == "ExternalInput":
            if name != partition_name:
                in_names.append(name)
        elif alloc.kind == "ExternalOutput":
            out_names.append(name)
            out_avals.append(
                jax.core.ShapedArray(tuple(alloc.tensor_shape), _mybir.dt.np(alloc.dtype))
            )
    n_params = len(in_names)
    all_in_names = in_names + out_names
    if partition_name is not None:
        all_in_names = all_in_names + [partition_name]

    def _body(*args):
        operands = list(args)
        if partition_name is not None:
            operands.append(bass2jax.partition_id_tensor())
        outs = bass2jax._bass_exec_p.bind(
            *operands,
            out_avals=tuple(out_avals),
            in_names=tuple(all_in_names),
            out_names=tuple(out_names),
            lowering_input_output_aliases=(),
            sim_require_finite=True,
            sim_require_nnan=True,
            nc=nc,
        )
        return tuple(outs)

    devices = jax.devices()[:N_CORES]
    mesh = Mesh(np.asarray(devices), ("core",))
    spec = PartitionSpec("core")
    n_outs = len(out_names)
    fn = jax.jit(
        shard_map(
            _body,
            mesh=mesh,
            in_specs=(spec,) * (n_params + n_outs),
            out_specs=(spec,) * n_outs,
            check_rep=False,
        ),
        donate_argnums=tuple(range(n_params, n_params + n_outs)),
        keep_unused=True,
    )
    sharding = NamedSharding(mesh, spec)
    zero_shapes = [(N_CORES * a.shape[0], *a.shape[1:]) for a in out_avals]
    zero_dtypes = [a.dtype for a in out_avals]

    def make_zeros():
        return [
            jax.jit(
                lambda shp=shp, dt=dt: jnp.zeros(shp, dt), out_shardings=sharding
            )()
            for shp, dt in zip(zero_shapes, zero_dtypes)
        ]

    runner = {
        "fn": fn,
        "in_names": in_names,
        "out_names": out_names,
        "out_avals": out_avals,
        "sharding": sharding,
        "make_zeros": make_zeros,
        "jax": jax,
    }
    _CACHE[rkey] = runner
    return runner


def _upload(in_maps):
    import jax

    r = _get_runner()
    concat = [
        np.concatenate([np.asarray(in_maps[c][name]) for c in range(N_CORES)], axis=0)
        for name in r["in_names"]
    ]
    dev = [jax.device_put(a, r["sharding"]) for a in concat]
    jax.block_until_ready(dev)
    _CACHE["dev_inputs"] = dev
    return dev


def _run_once():
    r = _get_runner()
    dev = _CACHE["dev_inputs"]
    outs = r["fn"](*dev, *r["make_zeros"]())
    r["jax"].block_until_ready(outs)
    return outs


def run_timed(iters: int = 10) -> list[float]:
    import time

    r = _get_runner()
    times = []
    for _ in range(iters):
        zeros = r["make_zeros"]()
        r["jax"].block_until_ready(zeros)
        t0 = time.perf_counter()
        outs = r["fn"](*_CACHE["dev_inputs"], *zeros)
        r["jax"].block_until_ready(outs)
        times.append(time.perf_counter() - t0)
    return times


def _marginal_call_s(r, n_hi: int = 8, trials: int = 3) -> float:
    """Marginal wall time per additional pipelined call of r["fn"]."""
    import time

    dev = _CACHE["dev_inputs"]
    jx = r["jax"]

    def total(n):
        zs = [r["make_zeros"]() for _ in range(n)]
        jx.block_until_ready(zs)
        t0 = time.perf_counter()
        outs = [r["fn"](*dev, *z) for z in zs]
        jx.block_until_ready(outs)
        return time.perf_counter() - t0

    total(1)  # warm executable + NEFF load
    best = None
    for _ in range(trials):
        m = (total(n_hi) - total(1)) / (n_hi - 1)
        best = m if best is None else min(best, m)
    return best


def measure_exec_ns(k_hi: int = 16, k_lo: int = 4) -> float:
    """Per-execution device time, low-noise: difference of marginal pipelined
    call costs between a k_hi-repeat NEFF and a k_lo-repeat NEFF of the same
    kernel. Dispatch and per-call overheads cancel; the lever arm is
    (k_hi - k_lo) executions (~4.5 ms), far above the wall-clock jitter of a
    single dispatch."""
    m_hi = _marginal_call_s(_get_runner(k_hi))
    m_lo = _marginal_call_s(_get_runner(k_lo))
    return (m_hi - m_lo) / (k_hi - k_lo) * 1e9


def kernel(hidden_states, cos, sin, attention_mask, wq, wk, wv, wo) -> np.ndarray:
    r = _get_runner()
    in_maps = make_in_maps(hidden_states, cos, sin, wq, wk, wv, wo, S)
    _upload(in_maps)
    outs = _run_once()
    arr = np.asarray(outs[0]).reshape(N_CORES, S, D)
    full = np.sum(arr, axis=0, dtype=np.float32)
    return full.reshape(B, S, D).astype(np.float32)


# revision 27
# speedup vs baseline: 1.0500x; 1.0500x over previous
"""Mixtral attention block (B=1, S=2048, D=4096, H=32, KVH=8, HD=128) on 8
Trainium2 NeuronCores.

Sharding: tensor-parallel by heads. Core c owns q-heads [4c, 4c+4) (columns
512c:512c+512 of wq), kv-head c (columns 128c:128c+128 of wk/wv) and the
matching rows 512c:512c+512 of wo. Each core computes a partial (S, D) output
through its wo row-slice; the host sums the 8 partials (the unshard step for
row-parallel wo) and reshapes to (1, S, D).

Device kernel layout choices (per core):
  - hidden_states is passed transposed (xT, [D, S]) in bf16 so all matmuls
    have the contraction dim on partitions.
  - Phase 1 computes Q^T [hd, S] / K^T [hd, S] (weight chunk stationary) and
    V in natural [S, hd] layout (xT chunk stationary), RoPE applied on the
    fp32 PSUM accumulators.
  - Phase 2 computes scores transposed, S^T[k, q] = (K^T chunk).T @ Q^T, so
    the softmax reduction over k is a partition-dim reduction done on the PE
    with an all-ones [128,128] stationary operand, which also broadcasts the
    denominators to all partitions; exp on ACT (scale folded in), causal
    masking by multiplying the exp'd diagonal-block tiles with 0/1 masks.
    P^T tiles then feed PV matmuls with V-natural stationary, producing
    A^T [hd, q] directly.
  - Phase 3 multiplies A^T by the broadcast reciprocal denominators and
    feeds wo row-chunks (A^T chunk stationary), accumulating the partial
    output in natural [S, D] layout.

bf16 matmuls with fp32 accumulation everywhere; exp and reciprocal in fp32.
No softmax max-subtraction: scores are O(10) for this problem family, far
from fp32 exp overflow, and softmax is shift-invariant.
attention_mask is all-ones per the problem spec (fill: "ones"), so only the
causal mask is applied.
"""

import sys

if "/opt/trn_rl_repo" not in sys.path:
    sys.path.insert(0, "/opt/trn_rl_repo")

import ast
from contextlib import ExitStack

import numpy as np
import ml_dtypes

import bass_rust
import concourse.bass as bass
import concourse.tile as tile
from concourse import mybir
from concourse.vector_clock import ScopedClock

BF16 = mybir.dt.bfloat16
F32 = mybir.dt.float32

B, S, D = 1, 2048, 4096
H, KVH, HD = 32, 8, 128
N_CORES = 8
QH = H // N_CORES          # 4 q heads per core
LOCAL = QH * HD            # 512 columns of wq / rows of wo per core
SCALE = float(HD) ** -0.5
SC = 512                   # sequence chunk for phase-1/2 free dims
N_SC = S // SC             # 4
N_D = D // 128             # 32 contraction chunks
N_ST = S // 128            # 16 sequence tiles of 128
N_OC = D // 512            # 8 output-column chunks


# ---------------------------------------------------------------------------
# The neuronxcc walrus build used under axon rejects instructions with more
# than one sync wait ("Too many sync wait commands" in setupSyncWait for the
# tail InstDrain). Split the TileContext tail drain into one drain per
# waited-on logical processor.
# ---------------------------------------------------------------------------
def _clock_values(vc) -> list[int]:
    r = repr(vc)
    return ast.literal_eval(r[r.index("[") : r.index("]") + 1])


def _split_drain_and_barrier(self, tick_clock, wait_clock):
    nc = self.nc
    gc = tick_clock.global_clock
    vals = _clock_values(gc)
    nz = [i for i, v in enumerate(vals) if v > 0]
    for i in nz or [None]:
        mask = [0] * len(vals)
        if i is not None:
            mask[i] = vals[i]
        part = gc.elementwise_min(bass_rust.VectorClock(mask))
        drain_inst = nc.sync.drain()
        wait_clock.add_sem_waits(drain_inst.ins, ScopedClock({None: part}))

    nc.all_engine_barrier()
    assert self.sems is not None
    popped = nc._tile_sem_poison_stack.pop()
    assert popped is self._sem_poison
    nc.clear_and_free_semaphores(list(self.sems.allocated().values()))
    nc.all_engine_barrier()


_orig_commit = tile.TileContext._commit_instruction


def _commit_split_waits(self, inst, lazy_reg_writes: bool = True):
    """This ISA build allows at most one sync wait per instruction
    (bass_rust.inst_waits_full). Tile's wait-assignment pass can attach
    several; spill the extras onto same-engine no-ops committed just before,
    which preserves per-engine ordering semantics."""
    si = getattr(inst, "sync_info", None)
    if si is not None and si.on_wait and len(si.on_wait) > 1:
        waits = list(si.on_wait)
        for w in waits[:-1]:
            nop = mybir.InstNoOp(
                name=self.nc.get_next_instruction_name(),
                sync_info=mybir.SyncInfo(on_wait=[w], on_update=[]),
                bass_nofuse=True,
                engine=inst.engine,
            )
            _orig_commit(self, nop, lazy_reg_writes)
        inst.sync_info = mybir.SyncInfo(
            on_wait=[waits[-1]], on_update=list(si.on_update)
        )
    return _orig_commit(self, inst, lazy_reg_writes)


def _apply_tile_patch():
    tile.TileContext._drain_and_barrier = _split_drain_and_barrier
    tile.TileContext._commit_instruction = _commit_split_waits


# ---------------------------------------------------------------------------
# Device kernel
# ---------------------------------------------------------------------------
def build_nc(seq: int = S, reps: int = 1) -> bass.Bass:
    """reps > 1 repeats the whole computation (including input DMAs) inside
    one NEFF; used to measure per-execution device time by differencing
    single-NEFF wall times, cancelling per-execution dispatch overhead.

    Structure: one fused loop over 512-wide sequence chunks. Iteration sc
    runs (a) projections+RoPE for chunk sc, (b) attention for q-chunk sc
    (all local heads; causally it needs only k/v chunks <= sc, all already
    computed), (c) the output projection for rows [512*sc, 512*sc+512).
    Interleaving keeps the PE instruction stream dense (PE executes in
    program order) and overlaps the xT/weight DMAs of later chunks with
    attention compute. PSUM is managed as one 8-slot pool of [128,512]
    fp32 banks shared by all stages."""
    _apply_tile_patch()
    n_sc = seq // SC

    nc = bass.Bass()
    xT = nc.declare_dram_parameter("xT", [D, seq], BF16, isOutput=False)
    wq = nc.declare_dram_parameter("wq", [D, LOCAL], BF16, isOutput=False)
    wk = nc.declare_dram_parameter("wk", [D, HD], BF16, isOutput=False)
    wv = nc.declare_dram_parameter("wv", [D, HD], BF16, isOutput=False)
    wo = nc.declare_dram_parameter("wo", [LOCAL, D], BF16, isOutput=False)
    cosT = nc.declare_dram_parameter("cosT", [HD, seq], F32, isOutput=False)
    sinT = nc.declare_dram_parameter("sinT", [HD, seq], F32, isOutput=False)
    # 0/1 masks for the 4 relative diagonal k-chunks of a 512-wide q chunk:
    # mask01[k, i, q] = 1 if q >= 128*i + k else 0
    mask01 = nc.declare_dram_parameter("mask01", [HD, 4, SC], BF16, isOutput=False)
    ident = nc.declare_dram_parameter("ident", [128, 128], BF16, isOutput=False)
    out = nc.declare_dram_parameter("out", [seq, D], F32, isOutput=True)

    with ExitStack() as ctx:
        tc = ctx.enter_context(tile.TileContext(nc))

        res = ctx.enter_context(tc.tile_pool(name="resident", bufs=1))
        wq_sb = res.tile([128, N_D, LOCAL], BF16, tag="wq")
        wk_sb = res.tile([128, N_D, HD], BF16, tag="wk")
        wv_sb = res.tile([128, N_D, HD], BF16, tag="wv")
        cos_sb = res.tile([HD, seq], F32, tag="cos")
        sin_sb = res.tile([HD, seq], F32, tag="sin")
        mask_sb = res.tile([HD, 4, SC], BF16, tag="mask")
        ones_sb = res.tile([128, 128], BF16, tag="ones")
        id_sb = res.tile([128, 128], BF16, tag="ident")
        qt_sb = [res.tile([HD, seq], BF16, tag=f"qt{j}", name=f"qt{j}") for j in range(QH)]
        kt_sb = res.tile([HD, seq], BF16, tag="kt")
        v_sb = res.tile([128, seq // 128, HD], BF16, tag="v")
        at_sb = [res.tile([HD, seq], BF16, tag=f"at{j}", name=f"at{j}") for j in range(QH)]
        wo_sb = res.tile([128, QH, D], BF16, tag="wo")

        xp = ctx.enter_context(tc.tile_pool(name="xp", bufs=1))
        tp = ctx.enter_context(tc.tile_pool(name="tp", bufs=3))
        pp = ctx.enter_context(tc.tile_pool(name="pp", bufs=3))
        op = ctx.enter_context(tc.tile_pool(name="op", bufs=3))
        ps = ctx.enter_context(tc.tile_pool(name="ps", bufs=8, space="PSUM"))

        wq_r = wq.rearrange("(n p) m -> p n m", p=128)
        wk_r = wk.rearrange("(n p) m -> p n m", p=128)
        wv_r = wv.rearrange("(n p) m -> p n m", p=128)
        wo_r = wo.rearrange("(n p) m -> p n m", p=128)
        x_r = xT.rearrange("(n p) m -> p n m", p=128)

        for _rep in range(reps):
            nc.vector.memset(ones_sb[:], 1.0)

            for sc in range(n_sc):
                # ---- stage A: projections + RoPE for chunk sc ----
                xh = xp.tile([128, N_D, SC], BF16, tag="xh")
                xg = 1 if sc == 0 else 4
                def load_xh_group(g):
                    nc.sync.dma_start(
                        xh[:, g : g + xg, :],
                        x_r[:, g : g + xg, sc * SC : (sc + 1) * SC],
                    )
                if sc > 0:
                    for g in range(0, N_D, xg):
                        load_xh_group(g)

                nc.sync.dma_start(
                    cos_sb[:, sc * SC : (sc + 1) * SC],
                    cosT[:, sc * SC : (sc + 1) * SC],
                )
                nc.sync.dma_start(
                    sin_sb[:, sc * SC : (sc + 1) * SC],
                    sinT[:, sc * SC : (sc + 1) * SC],
                )
                if sc == 0 and _rep == 0:
                    nc.sync.dma_start(mask_sb[:], mask01[:])
                    nc.sync.dma_start(id_sb[:], ident[:])

                q_ps = [ps.tile([128, SC], F32, tag="ps", name=f"qps{j}") for j in range(QH)]
                k_ps = ps.tile([128, SC], F32, tag="ps")
                v_ps = ps.tile([128, SC], F32, tag="ps")
                for d in range(N_D):
                    if sc == 0 and d % xg == 0:
                        load_xh_group(d)
                    if sc == 0 and _rep == 0:
                        # first use of each weight chunk: load it just in time
                        nc.sync.dma_start(wq_sb[:, d, :], wq_r[:, d, :])
                        if d % 4 == 0:
                            nc.sync.dma_start(
                                wk_sb[:, d : d + 4, :], wk_r[:, d : d + 4, :]
                            )
                            nc.sync.dma_start(
                                wv_sb[:, d : d + 4, :], wv_r[:, d : d + 4, :]
                            )
                    xt = xh[:, d, :]
                    first, last = d == 0, d == N_D - 1
                    for j in range(QH):
                        nc.tensor.matmul(
                            q_ps[j][:],
                            wq_sb[:, d, j * HD : (j + 1) * HD],
                            xt,
                            start=first,
                            stop=last,
                        )
                    nc.tensor.matmul(
                        k_ps[:], wk_sb[:, d, :], xt, start=first, stop=last
                    )
                    nc.tensor.matmul(
                        v_ps[:], wv_sb[:, d, :], xt, start=first, stop=last
                    )

                if sc == 0 and _rep == 0:
                    for j in range(QH):
                        nc.sync.dma_start(wo_sb[:, j, :], wo_r[:, j, :])

                # v_ps holds V^T [hd, s]; copy out to SBUF immediately (ahead
                # of the RoPE DVE work) so the PE transposes that follow the
                # projection matmuls in the PE stream aren't stalled, then
                # transpose 128x128 blocks on the PE into natural [s, hd].
                vt = pp.tile([128, SC], BF16, tag="vt")
                nc.scalar.copy(vt[:], v_ps[:])
                for m in range(4):
                    vtr = ps.tile([128, HD], BF16, tag="ps", name="vtr")
                    nc.tensor.transpose(
                        vtr[:], vt[:, m * HD : (m + 1) * HD], id_sb[:]
                    )
                    nc.scalar.copy(v_sb[:, sc * 4 + m, :], vtr[:])

                # RoPE. Stage B's first matmuls need only the fresh q0 chunk
                # (early kc use k/v chunks from previous iterations), so do q0
                # first, then k (needed by the diagonal kc late in the loop).
                cs = cos_sb[:, sc * SC : (sc + 1) * SC]
                sn = sin_sb[:, sc * SC : (sc + 1) * SC]
                for src_ps, dst in [(q_ps[0], qt_sb[0]), (k_ps, kt_sb)] + [
                    (q_ps[j], qt_sb[j]) for j in range(1, QH)
                ]:
                    t1 = tp.tile([HD, SC], F32, tag="t1")
                    t2 = tp.tile([HD, SC], F32, tag="t2")
                    dv = dst[:, sc * SC : (sc + 1) * SC]
                    nc.vector.tensor_mul(t1[:], src_ps[:], cs)
                    nc.vector.tensor_mul(t2[0:64, :], src_ps[64:128, :], sn[0:64, :])
                    nc.vector.tensor_mul(t2[64:128, :], src_ps[0:64, :], sn[64:128, :])
                    nc.gpsimd.tensor_sub(dv[0:64, :], t1[0:64, :], t2[0:64, :])
                    nc.gpsimd.tensor_add(dv[64:128, :], t1[64:128, :], t2[64:128, :])

                # ---- stage B: attention for q-chunk sc, all local heads ----
                qc = sc
                n_k = 4 * qc + 4
                for j in range(QH):
                    ssum = ps.tile([128, SC], F32, tag="ps")
                    pv = ps.tile([128, SC], F32, tag="ps")
                    qt = qt_sb[j][:, qc * SC : (qc + 1) * SC]
                    for kc in range(n_k):
                        st = ps.tile([128, SC], F32, tag="ps")
                        nc.tensor.matmul(
                            st[:],
                            kt_sb[:, kc * 128 : (kc + 1) * 128],
                            qt,
                            start=True,
                            stop=True,
                        )
                        pt = pp.tile([128, SC], BF16, tag="pt")
                        nc.scalar.activation(
                            pt[:], st[:], mybir.ActivationFunctionType.Exp,
                            scale=SCALE,
                        )
                        if kc >= 4 * qc:
                            nc.vector.tensor_mul(
                                pt[:], pt[:], mask_sb[:, kc - 4 * qc, :]
                            )
                        first, last = kc == 0, kc == n_k - 1
                        nc.tensor.matmul(
                            ssum[:], ones_sb[:], pt[:], start=first, stop=last
                        )
                        nc.tensor.matmul(
                            pv[:], v_sb[:, kc, :], pt[:], start=first, stop=last
                        )
                    recip = pp.tile([128, SC], F32, tag="recip")
                    nc.vector.reciprocal(recip[:], ssum[:])
                    nc.vector.tensor_mul(
                        at_sb[j][:, qc * SC : (qc + 1) * SC], pv[:], recip[:]
                    )

                # ---- stage C: output projection for rows of chunk sc ----
                for st_i in range(4 * sc, 4 * sc + 4):
                    for oc in range(N_OC):
                        ops = ps.tile([128, 512], F32, tag="ps")
                        for j in range(QH):
                            nc.tensor.matmul(
                                ops[:],
                                at_sb[j][:, st_i * 128 : (st_i + 1) * 128],
                                wo_sb[:, j, oc * 512 : (oc + 1) * 512],
                                start=(j == 0),
                                stop=(j == QH - 1),
                            )
                        osb = op.tile([128, 512], F32, tag="osb")
                        nc.vector.tensor_copy(osb[:], ops[:])
                        nc.sync.dma_start(
                            out[st_i * 128 : (st_i + 1) * 128, oc * 512 : (oc + 1) * 512],
                            osb[:],
                        )
    return nc


# ---------------------------------------------------------------------------
# Host-side input prep
# ---------------------------------------------------------------------------
def make_masks() -> np.ndarray:
    k = np.arange(HD)[:, None]
    q = np.arange(SC)[None, :]
    m = np.stack([(q >= 128 * i + k) for i in range(4)], axis=1).astype(ml_dtypes.bfloat16)
    return m


def make_in_maps(hidden_states, cos, sin, wq, wk, wv, wo, seq: int = S):
    bf = ml_dtypes.bfloat16
    x = np.asarray(hidden_states, np.float32).reshape(seq, D)
    xT = np.ascontiguousarray(x.T).astype(bf)
    cosT = np.ascontiguousarray(np.asarray(cos, np.float32).reshape(seq, HD).T)
    sinT = np.ascontiguousarray(np.asarray(sin, np.float32).reshape(seq, HD).T)
    masks = make_masks()
    wqf = np.asarray(wq, np.float32)
    wkf = np.asarray(wk, np.float32)
    wvf = np.asarray(wv, np.float32)
    wof = np.asarray(wo, np.float32)
    in_maps = []
    for c in range(N_CORES):
        in_maps.append(
            {
                "xT": xT,
                "wq": np.ascontiguousarray(wqf[:, c * LOCAL : (c + 1) * LOCAL]).astype(bf),
                "wk": np.ascontiguousarray(wkf[:, c * HD : (c + 1) * HD]).astype(bf),
                "wv": np.ascontiguousarray(wvf[:, c * HD : (c + 1) * HD]).astype(bf),
                "wo": np.ascontiguousarray(wof[c * LOCAL : (c + 1) * LOCAL, :]).astype(bf),
                "cosT": cosT,
                "sinT": sinT,
                "mask01": masks,
                "ident": np.eye(128, dtype=np.float32).astype(bf),
            }
        )
    return in_maps


_CACHE: dict = {}


def _get_runner(reps: int = 1):
    """Compile the device program once; return a callable over device-resident
    sharded inputs. Mirrors concourse.bass2jax.run_bass_via_pjrt's multi-core
    branch, but keeps the jitted executable and lets us re-run for timing."""
    rkey = f"runner{reps}"
    if rkey in _CACHE:
        return _CACHE[rkey]

    import jax
    import jax.numpy as jnp
    from jax.experimental.shard_map import shard_map
    from jax.sharding import Mesh, NamedSharding, PartitionSpec
    from concourse import bass2jax, mybir as _mybir

    bass2jax.install_neuronx_cc_hook()
    nc = _CACHE.setdefault(f"nc{reps}", build_nc(S, reps))

    partition_name = nc.partition_id_tensor.name if nc.partition_id_tensor else None
    in_names: list[str] = []
    out_names: list[str] = []
    out_avals: list = []
    for alloc in nc.m.functions[0].allocations:
        if not isinstance(alloc, _mybir.MemoryLocationSet):
            continue
        name = alloc.memorylocations[0].name
        if alloc.kind == "ExternalInput":
            if name != partition_name:
                in_names.append(name)
        elif alloc.kind == "ExternalOutput":
            out_names.append(name)
            out_avals.append(
                jax.core.ShapedArray(tuple(alloc.tensor_shape), _mybir.dt.np(alloc.dtype))
            )
    n_params = len(in_names)
    all_in_names = in_names + out_names
    if partition_name is not None:
        all_in_names = all_in_names + [partition_name]

    def _body(*args):
        operands = list(args)
        if partition_name is not None:
            operands.append(bass2jax.partition_id_tensor())
        outs = bass2jax._bass_exec_p.bind(
            *operands,
            out_avals=tuple(out_avals),
            in_names=tuple(all_in_names),
            out_names=tuple(out_names),
            lowering_input_output_aliases=(),
            sim_require_finite=True,
            sim_require_nnan=True,
            nc=nc,
        )
        return tuple(outs)

    devices = jax.devices()[:N_CORES]
    mesh = Mesh(np.asarray(devices), ("core",))
    spec = PartitionSpec("core")
    n_outs = len(out_names)
    fn = jax.jit(
        shard_map(
            _body,
            mesh=mesh,
            in_specs=(spec,) * (n_params + n_outs),
            out_specs=(spec,) * n_outs,
            check_rep=False,
        ),
        donate_argnums=tuple(range(n_params, n_params + n_outs)),
        keep_unused=True,
    )
    sharding = NamedSharding(mesh, spec)
    zero_shapes = [(N_CORES * a.shape[0], *a.shape[1:]) for a in out_avals]
    zero_dtypes = [a.dtype for a in out_avals]

    def make_zeros():
        return [
            jax.jit(
                lambda shp=shp, dt=dt: jnp.zeros(shp, dt), out_shardings=sharding
            )()
            for shp, dt in zip(zero_shapes, zero_dtypes)
        ]

    runner = {
        "fn": fn,
        "in_names": in_names,
        "out_names": out_names,
        "out_avals": out_avals,
        "sharding": sharding,
        "make_zeros": make_zeros,
        "jax": jax,
    }
    _CACHE[rkey] = runner
    return runner


def _upload(in_maps):
    import jax

    r = _get_runner()
    concat = [
        np.concatenate([np.asarray(in_maps[c][name]) for c in range(N_CORES)], axis=0)
        for name in r["in_names"]
    ]
    dev = [jax.device_put(a, r["sharding"]) for a in concat]
    jax.block_until_ready(dev)
    _CACHE["dev_inputs"] = dev
    return dev


def _run_once():
    r = _get_runner()
    dev = _CACHE["dev_inputs"]
    outs = r["fn"](*dev, *r["make_zeros"]())
    r["jax"].block_until_ready(outs)
    return outs


def run_timed(iters: int = 10) -> list[float]:
    import time

    r = _get_runner()
    times = []
    for _ in range(iters):
        zeros = r["make_zeros"]()
        r["jax"].block_until_ready(zeros)
        t0 = time.perf_counter()
        outs = r["fn"](*_CACHE["dev_inputs"], *zeros)
        r["jax"].block_until_ready(outs)
        times.append(time.perf_counter() - t0)
    return times


def _marginal_call_s(r, n_hi: int = 16, trials: int = 5) -> float:
    """Marginal wall time per additional pipelined call of r["fn"]
    (median over trials of a large-lever slope)."""
    import statistics, time

    dev = _CACHE["dev_inputs"]
    jx = r["jax"]

    def total(n):
        zs = [r["make_zeros"]() for _ in range(n)]
        jx.block_until_ready(zs)
        t0 = time.perf_counter()
        outs = [r["fn"](*dev, *z) for z in zs]
        jx.block_until_ready(outs)
        return time.perf_counter() - t0

    total(1)  # warm executable + NEFF load
    diffs = [(total(n_hi) - total(1)) / (n_hi - 1) for _ in range(trials)]
    return statistics.median(diffs)


def measure_exec_ns(k_hi: int = 16, k_lo: int = 4) -> float:
    """Per-execution device time, low-noise: difference of marginal pipelined
    call costs between a k_hi-repeat NEFF and a k_lo-repeat NEFF of the same
    kernel. Dispatch and per-call overheads cancel; the lever arm is
    (k_hi - k_lo) executions (~4.5 ms), far above the wall-clock jitter of a
    single dispatch."""
    m_hi = _marginal_call_s(_get_runner(k_hi))
    m_lo = _marginal_call_s(_get_runner(k_lo))
    return (m_hi - m_lo) / (k_hi - k_lo) * 1e9


def kernel(hidden_states, cos, sin, attention_mask, wq, wk, wv, wo) -> np.ndarray:
    r = _get_runner()
    in_maps = make_in_maps(hidden_states, cos, sin, wq, wk, wv, wo, S)
    _upload(in_maps)
    outs = _run_once()
    arr = np.asarray(outs[0]).reshape(N_CORES, S, D)
    full = np.sum(arr, axis=0, dtype=np.float32)
    return full.reshape(B, S, D).astype(np.float32)
